# revision 26
# baseline (speedup 1.0000x reference)
"""Trainium2 Bass kernel for ContinuousAxialDW.

The reference op (continuous-offset axial depthwise conv, bilinear sampling)
collapses to two 1D depthwise convolutions with *integer* shifts, because the
bilinear fraction frac(off*r) is constant along the sampled axis:

    out[b,c,h,w] = x + sum_s A[c,s]*x[b,c,h+s,w] + sum_t B[c,t]*x[b,c,h,w+t]

with zero padding at the borders.  Folding the identity into the H-term this
is, per channel c:

    out[b,c] = Mh[c] @ X  +  X @ Sw[c]        (X = x[b,c], 256x256)

where Mh = I + banded(A), Sw = banded(B) are host-built 256x256 banded
matrices.  Both terms run on the TensorEngine:

  * term1 = Mh @ X:      matmul(lhsT=Mh^T chunk, rhs=X chunk)    [no transpose]
  * term2 = X @ Sw:      matmul(lhsT=(X^T) chunk, rhs=Sw chunk)  [X^T via PE
                         transpose; result lands directly in normal layout]

Both accumulate in the same PSUM bank, evacuated once by DVE/ACT.

Sharding: channels across the 8 cores (12 ch/core, all 8 batch images), so the
per-channel banded matrices are DMA'd once and reused across 8 images.
"""

import os
import sys

import numpy as np

for _p in ("/opt/trn_rl_repo", "/root/.axon_site/_ro/trn_rl_repo"):
    if _p not in sys.path and os.path.isdir(_p):
        sys.path.append(_p)

import concourse.bass as bass
import concourse.mybir as mybir
from concourse import bacc, tile
from concourse.bass_utils import run_bass_kernel_spmd

N_CORES = 8
B, C, H, W = 8, 96, 256, 256
C_LOC = C // N_CORES  # 12 channels per core
KTAPS = 7

F32 = mybir.dt.float32
F32R = mybir.dt.float32r
BF16 = mybir.dt.bfloat16

# run_bass_kernel_spmd results of the most recent kernel() call (for test
# harness introspection: exec_time_ns when BASS_TRACE=1).
LAST_RESULTS = None

_PROGRAM = None  # cached Bass program (input-independent)


def _emit(tc, x_d, m_d, i_d, o_d, bw):
    """Emit the per-core program.

    Per-core DRAM tensors:
      x_d: [C_LOC=12, 2, 128, B, W] input shard (all batches, 12 channels)
      m_d: [12, 128, 4, 256]  per-channel banded matrices, 4 chunks each:
           m=0,1: MhT rows 0:128 / 128:256   (lhsT for term1)
           m=2,3: Sw  rows 0:128 / 128:256   (rhs for term2)
      i_d: [128, 128] identity (for PE transposes)
      o_d: [C_LOC, 2, 128, B, W] output shard
      bw:  band halfwidth of the Sw/Mh Toeplitz matrices (5 for r=1.5).
           term2 exploits it: out cols [0,128-bw) need only X^T rows [0,128)
           and cols [128+bw,256) only rows [128,256), so those matmuls
           stream 123 cols against a single K=128 window instead of 256
           cols against both windows; only the 2*bw straddle cols need two
           (tiny) matmuls.  Halves term2 PE time.
    """
    nc = tc.nc
    n_pairs = 4 * C_LOC  # global pair index g = c*4 + p
    lo = 128 - bw  # last n-chunk boundary before the straddle
    hi = 128 + bw
    mcols = 512 + 2 * bw  # compressed per-channel matrix strip width
    with (
        tc.tile_pool(name="const", bufs=1) as cpool,
        tc.tile_pool(name="mats", bufs=3) as mpool,
        tc.tile_pool(name="xin", bufs=3) as xpool,
        tc.tile_pool(name="xtp", bufs=4) as xtpool,
        tc.tile_pool(name="outp", bufs=2) as opool,
        tc.tile_pool(name="psx", bufs=4, space="PSUM") as psx,
        tc.tile_pool(name="pso", bufs=4, space="PSUM") as pso,
    ):
        ident = cpool.tile([128, 128], BF16, name="ident")
        nc.sync.dma_start(ident[:], i_d[:])

        chans = {}  # c -> (mat, xh[2], oh[2])
        pairs = {}  # g -> (pxt[2], xt[2])
        outs = {}  # g -> po[2]

        def start_channel(c):
            # all loads on SP (HWDGE); stores go to other engines so SP never
            # stalls on compute and the load pipeline runs ahead.  DRAM layouts
            # are host-pre-shuffled so every DMA is contiguous per partition.
            mat = mpool.tile([128, mcols], BF16, name=f"mat{c}", tag="mat")
            xh = [
                xpool.tile([128, 2048], BF16, name=f"x{hb}_{c}", tag=f"x{hb}")
                for hb in range(2)
            ]
            if c == 0:
                # split the very first loads by batch-half (and issue x
                # before mat) so pair 0's transposes start as soon as the
                # b0..3 halves land: shaves the pipeline fill
                for hb in range(2):
                    nc.sync.dma_start(xh[hb][:, 0:1024], x_d[c, hb, :, 0:4, :])
                for hb in range(2):
                    nc.sync.dma_start(xh[hb][:, 1024:2048], x_d[c, hb, :, 4:8, :])
                nc.sync.dma_start(mat[:], m_d[c])
            else:
                nc.sync.dma_start(mat[:], m_d[c])
                for hb in range(2):
                    nc.sync.dma_start(xh[hb][:], x_d[c, hb])
            oh = [
                opool.tile([128, 2048], BF16, name=f"o{hb}_{c}", tag=f"o{hb}")
                for hb in range(2)
            ]
            chans[c] = (mat, xh, oh)

        def tr_half(g, wb):
            # 4 PE transposes building pxt[wb] = X^T w-block for pair g,
            # then its evacuation (DVE for wb=0, ACT for wb=1).
            c, p = divmod(g, 4)
            _, xh, _ = chans[c]
            if wb == 0:
                pairs[g] = ([None, None], [None, None])
            pxt, xt = pairs[g]
            pxt[wb] = psx.tile([128, 512], BF16, name=f"pxt{wb}_{g}", tag="pxt")
            for bi in range(2):
                b = 2 * p + bi
                for hb in range(2):
                    nc.tensor.transpose(
                        pxt[wb][:, bi * 256 + hb * 128 : bi * 256 + hb * 128 + 128],
                        xh[hb][:, b * 256 + wb * 128 : b * 256 + wb * 128 + 128],
                        ident[:],
                    )
            xt[wb] = xtpool.tile([128, 512], BF16, name=f"xt{wb}_{g}", tag="xt")
            if wb == 0:
                nc.vector.tensor_copy(xt[wb][:], pxt[wb][:])
            else:
                nc.scalar.copy(xt[wb][:], pxt[wb][:])

        def terms_group(g, hb):
            # the matmul accumulation group for po[hb] of pair g
            c, p = divmod(g, 4)
            mat, xh, oh = chans[c]
            _, xt = pairs[g]
            if hb == 0:
                outs[g] = [None, None]
            po = outs[g]
            po[hb] = pso.tile([128, 512], F32, name=f"po{hb}_{g}", tag="po")
            # term1: Mh @ X, N=512.  Compressed mat strip: cols [0,128) hold
            # the (Toeplitz-shared) diagonal block MhT[kb*128:.., hb*128:..]
            # for kb==hb; cols [128,256) pack both off-diagonal corners
            # (nonzero only in a bw x bw corner): rows [0,64) the (kb=1,hb=0)
            # block's top rows, rows [64,128) the (kb=0,hb=1) block's bottom
            # rows, so the corner matmuls are K=64 at base partition 0/64.
            nc.tensor.matmul(
                po[hb][:],
                lhsT=mat[:, 0:128],
                rhs=xh[hb][:, p * 512 : p * 512 + 512],
                start=True,
                stop=False,
            )
            if hb == 0:
                nc.tensor.matmul(
                    po[0][:],
                    lhsT=mat[0:64, 128:256],
                    rhs=xh[1][0:64, p * 512 : p * 512 + 512],
                    start=False,
                    stop=False,
                )
            else:
                nc.tensor.matmul(
                    po[1][:],
                    lhsT=mat[64:128, 128:256],
                    rhs=xh[0][64:128, p * 512 : p * 512 + 512],
                    start=False,
                    stop=False,
                )

            # term2: X @ Sw, banded.  X^T rows [0,128) only reach out cols
            # [0, 128+bw) and rows [128,256) only cols [128-bw, 256), so per
            # image two K=128 matmuls of N=128+bw cover everything (the
            # straddle cols [lo,hi) accumulate from both); cols streamed drop
            # from 512 to 2*(128+bw) per (hb, bi).
            def xts(wb, bi):
                base = bi * 256 + hb * 128
                return xt[wb][:, base : base + 128]

            for bi in range(2):
                nc.tensor.matmul(
                    po[hb][:, bi * 256 : bi * 256 + hi],
                    lhsT=xts(0, bi),
                    rhs=mat[:, 256 : 256 + hi],
                    start=False,
                    stop=False,
                )
                nc.tensor.matmul(
                    po[hb][:, bi * 256 + lo : bi * 256 + 256],
                    lhsT=xts(1, bi),
                    rhs=mat[:, 256 + hi : 256 + 2 * hi],
                    start=False,
                    stop=(bi == 1),
                )

            # evacuate once the group is complete
            if hb == 0:
                nc.vector.tensor_copy(oh[0][:, p * 512 : p * 512 + 512], po[0][:])
            else:
                nc.scalar.copy(oh[1][:, p * 512 : p * 512 + 512], po[1][:])
            if hb == 1:
                del pairs[g], outs[g]
                if c == C_LOC - 1:
                    # last channel: store per pair so the tail is one chunk,
                    # not a whole channel; use sync (idle by now) + scalar so
                    # the slow gpsimd software-DGE drain is off the tail
                    nc.sync.dma_start(
                        o_d[c, 0, :, 2 * p : 2 * p + 2, :],
                        oh[0][:, p * 512 : p * 512 + 512],
                    )
                    nc.scalar.dma_start(
                        o_d[c, 1, :, 2 * p : 2 * p + 2, :],
                        oh[1][:, p * 512 : p * 512 + 512],
                    )
                elif p == 3:  # channel done: store (off the SP engine)
                    nc.gpsimd.dma_start(o_d[c, 0], oh[0][:])
                    nc.scalar.dma_start(o_d[c, 1], oh[1][:])

        # software pipeline: pair g's transposes are interleaved between pair
        # g-1's two matmul groups, so TensorE always has real matmuls in every
        # HAM window and the X^T evac latency is hidden one pair ahead.
        for g in range(n_pairs + 1):
            if g < n_pairs:
                c, p = divmod(g, 4)
                if p == 0:
                    start_channel(c)
                tr_half(g, 0)
            if g > 0:
                terms_group(g - 1, 0)
            if g < n_pairs:
                tr_half(g, 1)
            if g > 0:
                terms_group(g - 1, 1)


def _build_program(bw):
    global _PROGRAM
    if _PROGRAM is not None and _PROGRAM[0] == bw:
        return _PROGRAM[1]
    nc = bacc.Bacc("TRN2", target_bir_lowering=False, debug=False, num_devices=N_CORES)
    # DMA-native layouts (host pre-shuffles): x/out as [c, hb, h, b, w] so a
    # [128, 2048] tile load/store is contiguous 4KB per partition; mats as
    # [c, p, m, f] so a [128, 1024] tile load is contiguous 2KB per partition.
    x_d = nc.dram_tensor("x_sh", [C_LOC, 2, 128, B, W], BF16, kind="ExternalInput").ap()
    m_d = nc.dram_tensor(
        "mats", [C_LOC, 128, 512 + 2 * bw], BF16, kind="ExternalInput"
    ).ap()
    i_d = nc.dram_tensor("ident", [128, 128], BF16, kind="ExternalInput").ap()
    o_d = nc.dram_tensor("out_sh", [C_LOC, 2, 128, B, W], BF16, kind="ExternalOutput").ap()
    with tile.TileContext(nc) as tc:
        _emit(tc, x_d, m_d, i_d, o_d, bw)
    nc.compile()
    _PROGRAM = (bw, nc)
    return nc


def _eff_coeffs(taps, r):
    """taps: [k, C] per-tap depthwise weights -> dict integer_shift -> coeff[C].

    Mirrors the reference: pos = coord + off*r (f32), i0 = floor(pos),
    frac = pos - i0; both are constant per tap since coord is integral.
    """
    r_val = max(float(np.float32(r)), 1.0)
    k = taps.shape[0]
    pad = k // 2
    coeffs = {}
    for i, off in enumerate(range(-pad, pad + 1)):
        pos = np.float32(off * np.float32(r_val))
        s0 = int(np.floor(pos))
        f = float(np.float32(pos)) - s0
        for s, cmul in ((s0, 1.0 - f), (s0 + 1, f)):
            if cmul != 0.0:
                acc = coeffs.setdefault(s, np.zeros(taps.shape[1], np.float64))
                acc += cmul * taps[i].astype(np.float64)
    return coeffs


def _build_mats(weight_h, weight_w, r, bw):
    """Host-build the compressed per-channel banded-matrix strip.

    Returns [C, 128, 512+2*bw] f32, per channel:
      cols [0,128):        MhT diagonal block (Toeplitz: same for both
                           (kb,hb)=(0,0) and (1,1)), identity folded in
      cols [128,256):      both off-diagonal corners packed by row range:
                           rows [0,64) = MhT[128:192, 0:128]  (for hb=0),
                           rows [64,128) = MhT[64:128, 128:256] (for hb=1)
      cols [256,256+hi):   Sw[0:128, 0:hi)       (hi = 128+bw)
      cols [256+hi,...):   Sw[128:256, lo:256)   (lo = 128-bw), hi cols
    where MhT[h+s, h] = A[c, s] (+I) and Sw[w+t, w] = B[c, t].
    """
    lo, hi = 128 - bw, 128 + bw
    ch = _eff_coeffs(weight_h[:, 0, :, 0].T, r)
    cw = _eff_coeffs(weight_w[:, 0, 0, :].T, r)
    mh_t = np.zeros((C, H, H), np.float64)
    mh_t[:, np.arange(H), np.arange(H)] = 1.0
    for s, coef in ch.items():
        i = np.arange(max(0, s), H + min(0, s))
        mh_t[:, i, i - s] += coef[:, None]
    sw = np.zeros((C, W, W), np.float64)
    for t, coef in cw.items():
        i = np.arange(max(0, t), W + min(0, t))
        sw[:, i, i - t] += coef[:, None]
    mats = np.zeros((C, 128, 512 + 2 * bw), np.float32)
    mats[:, :, 0:128] = mh_t[:, 0:128, 0:128]
    mats[:, 0:64, 128:256] = mh_t[:, 128:192, 0:128]
    mats[:, 64:128, 128:256] = mh_t[:, 64:128, 128:256]
    mats[:, :, 256 : 256 + hi] = sw[:, 0:128, 0:hi]
    mats[:, :, 256 + hi : 256 + 2 * hi] = sw[:, 128:256, lo:256]
    return mats


def kernel(**inputs):
    global LAST_RESULTS
    x = np.ascontiguousarray(np.asarray(inputs["x"], dtype=np.float32))
    weight_h = np.asarray(inputs["weight_h"], dtype=np.float32)
    weight_w = np.asarray(inputs["weight_w"], dtype=np.float32)
    r = np.asarray(inputs["r"], dtype=np.float32)
    assert x.shape == (B, C, H, W), x.shape

    import ml_dtypes

    bf16 = ml_dtypes.bfloat16
    # band halfwidth: max integer shift is floor(3*r)+1 (bilinear upper tap)
    r_val = max(float(np.float32(r)), 1.0)
    bw = int(np.floor(3.0 * r_val)) + 1
    assert 1 <= bw <= 60, bw

    mats = _build_mats(weight_h, weight_w, r, bw)  # [C, 128, 512+2bw]
    mats = np.ascontiguousarray(mats.astype(bf16))
    ident = np.ascontiguousarray(np.eye(128, dtype=bf16))

    # [B, C, H, W] -> per-shard [C_LOC, 2(hb), 128(h), B, W] (DMA-native)
    xs = x.astype(bf16).transpose(1, 2, 0, 3).reshape(C, 2, 128, B, W)

    nc = _build_program(bw)
    in_maps = [
        {
            "x_sh": np.ascontiguousarray(xs[i * C_LOC : (i + 1) * C_LOC]),
            "mats": np.ascontiguousarray(mats[i * C_LOC : (i + 1) * C_LOC]),
            "ident": ident,
        }
        for i in range(N_CORES)
    ]
    res = run_bass_kernel_spmd(nc, in_maps, list(range(N_CORES)))
    LAST_RESULTS = res
    # [C_LOC, 2, 128, B, W] bf16 per core -> [B, C, H, W] f32
    o = np.concatenate(
        [np.asarray(res.results[i]["out_sh"]) for i in range(N_CORES)], axis=0
    )
    out = o.reshape(C, H, B, W).transpose(2, 0, 1, 3).astype(np.float32)
    return np.ascontiguousarray(out)



# revision 32
# speedup vs baseline: 1.2148x; 1.2148x over previous
"""Trainium2 Bass kernel for ContinuousAxialDW.

The reference op (continuous-offset axial depthwise conv, bilinear sampling)
collapses to two 1D depthwise convolutions with *integer* shifts, because the
bilinear fraction frac(off*r) is constant along the sampled axis:

    out[b,c,h,w] = x + sum_s A[c,s]*x[b,c,h+s,w] + sum_t B[c,t]*x[b,c,h,w+t]

with zero padding at the borders.  Folding the identity into the H-term this
is, per channel c:

    out[b,c] = Mh[c] @ X  +  X @ Sw[c]        (X = x[b,c], 256x256)

where Mh = I + banded(A), Sw = banded(B) are host-built 256x256 banded
matrices.  Both terms run on the TensorEngine:

  * term1 = Mh @ X:      matmul(lhsT=Mh^T chunk, rhs=X chunk)    [no transpose]
  * term2 = X @ Sw:      matmul(lhsT=(X^T) chunk, rhs=Sw chunk)  [X^T via PE
                         transpose; result lands directly in normal layout]

Both accumulate in the same PSUM bank, evacuated once by DVE/ACT.

Sharding: channels across the 8 cores (12 ch/core, all 8 batch images), so the
per-channel banded matrices are DMA'd once and reused across 8 images.
"""

import os
import sys

import numpy as np

for _p in ("/opt/trn_rl_repo", "/root/.axon_site/_ro/trn_rl_repo"):
    if _p not in sys.path and os.path.isdir(_p):
        sys.path.append(_p)

import concourse.bass as bass
import concourse.mybir as mybir
from concourse import bacc, tile
from concourse.bass_utils import run_bass_kernel_spmd

N_CORES = 8
B, C, H, W = 8, 96, 256, 256
C_LOC = C // N_CORES  # 12 channels per core
KTAPS = 7

F32 = mybir.dt.float32
F32R = mybir.dt.float32r
BF16 = mybir.dt.bfloat16

# run_bass_kernel_spmd results of the most recent kernel() call (for test
# harness introspection: exec_time_ns when BASS_TRACE=1).
LAST_RESULTS = None

_PROGRAM = None  # cached Bass program (input-independent)


def _emit(tc, x_d, m_d, i_d, o_d, bw):
    """Emit the per-core program.

    Per-core DRAM tensors:
      x_d: [C_LOC=12, 2, 128, B, W] input shard (all batches, 12 channels)
      m_d: [12, 128, 4, 256]  per-channel banded matrices, 4 chunks each:
           m=0,1: MhT rows 0:128 / 128:256   (lhsT for term1)
           m=2,3: Sw  rows 0:128 / 128:256   (rhs for term2)
      i_d: [128, 128] identity (for PE transposes)
      o_d: [C_LOC, 2, 128, B, W] output shard
      bw:  band halfwidth of the Sw/Mh Toeplitz matrices (5 for r=1.5).
           term2 exploits it: out cols [0,128-bw) need only X^T rows [0,128)
           and cols [128+bw,256) only rows [128,256), so those matmuls
           stream 123 cols against a single K=128 window instead of 256
           cols against both windows; only the 2*bw straddle cols need two
           (tiny) matmuls.  Halves term2 PE time.
    """
    nc = tc.nc
    n_pairs = 4 * C_LOC  # global pair index g = c*4 + p
    lo = 128 - bw  # last n-chunk boundary before the straddle
    hi = 128 + bw
    mcols = 640 + 2 * bw  # compressed per-channel matrix strip width
    with (
        tc.tile_pool(name="const", bufs=1) as cpool,
        tc.tile_pool(name="mats", bufs=3) as mpool,
        tc.tile_pool(name="xin", bufs=3) as xpool,
        tc.tile_pool(name="xtp", bufs=4) as xtpool,
        tc.tile_pool(name="outp", bufs=2) as opool,
        tc.tile_pool(name="psx", bufs=4, space="PSUM") as psx,
        tc.tile_pool(name="pso", bufs=4, space="PSUM") as pso,
    ):
        ident = cpool.tile([128, 128], BF16, name="ident")
        nc.sync.dma_start(ident[:], i_d[:])

        chans = {}  # c -> (mat, xh[2], oh[2])
        pairs = {}  # g -> (pxt[2], xt[2])
        outs = {}  # g -> po[2]

        def start_channel(c):
            # all loads on SP (HWDGE); stores go to other engines so SP never
            # stalls on compute and the load pipeline runs ahead.  DRAM layouts
            # are host-pre-shuffled so every DMA is contiguous per partition.
            mat = mpool.tile([128, mcols], BF16, name=f"mat{c}", tag="mat")
            xh = [
                xpool.tile([128, 2048], BF16, name=f"x{hb}_{c}", tag=f"x{hb}")
                for hb in range(2)
            ]
            if c == 0:
                # split the very first loads by batch-half (and issue x
                # before mat) so pair 0's transposes start as soon as the
                # b0..3 halves land: shaves the pipeline fill
                for hb in range(2):
                    nc.sync.dma_start(xh[hb][:, 0:1024], x_d[c, hb, :, 0:4, :])
                for hb in range(2):
                    nc.sync.dma_start(xh[hb][:, 1024:2048], x_d[c, hb, :, 4:8, :])
                nc.sync.dma_start(mat[:], m_d[c])
            else:
                nc.sync.dma_start(mat[:], m_d[c])
                for hb in range(2):
                    nc.sync.dma_start(xh[hb][:], x_d[c, hb])
            oh = [
                opool.tile([128, 2048], BF16, name=f"o{hb}_{c}", tag=f"o{hb}")
                for hb in range(2)
            ]
            chans[c] = (mat, xh, oh)

        def tr_half(g, wb):
            # 4 PE transposes building pxt[wb] = X^T w-block for pair g,
            # then its evacuation (DVE for wb=0, ACT for wb=1).
            c, p = divmod(g, 4)
            _, xh, _ = chans[c]
            if wb == 0:
                pairs[g] = ([None, None], [None, None])
            pxt, xt = pairs[g]
            pxt[wb] = psx.tile([128, 512], BF16, name=f"pxt{wb}_{g}", tag="pxt")
            for bi in range(2):
                b = 2 * p + bi
                for hb in range(2):
                    nc.tensor.transpose(
                        pxt[wb][:, bi * 256 + hb * 128 : bi * 256 + hb * 128 + 128],
                        xh[hb][:, b * 256 + wb * 128 : b * 256 + wb * 128 + 128],
                        ident[:],
                    )
            xt[wb] = xtpool.tile([128, 512], BF16, name=f"xt{wb}_{g}", tag="xt")
            if wb == 0:
                nc.vector.tensor_copy(xt[wb][:], pxt[wb][:])
            else:
                nc.scalar.copy(xt[wb][:], pxt[wb][:])

        def terms_group(g, hb):
            # the matmul accumulation group for po[hb] of pair g
            c, p = divmod(g, 4)
            mat, xh, oh = chans[c]
            _, xt = pairs[g]
            if hb == 0:
                outs[g] = [None, None]
            po = outs[g]
            po[hb] = pso.tile([128, 512], F32, name=f"po{hb}_{g}", tag="po")
            # term1: Mh @ X, N=512.  Compressed mat strip: cols [0,128) hold
            # the (Toeplitz-shared) diagonal block MhT[kb*128:.., hb*128:..]
            # for kb==hb; cols [128,256) / [256,384) the off-diagonal corner
            # blocks (nonzero only in a bw x bw corner; kept full K=128 —
            # K=64 sub-tile positions measurably slow the whole PE down)
            nc.tensor.matmul(
                po[hb][:],
                lhsT=mat[:, 0:128],
                rhs=xh[hb][:, p * 512 : p * 512 + 512],
                start=True,
                stop=False,
            )
            nc.tensor.matmul(
                po[hb][:],
                lhsT=mat[:, 128 + hb * 128 : 256 + hb * 128],
                rhs=xh[1 - hb][:, p * 512 : p * 512 + 512],
                start=False,
                stop=False,
            )

            # term2: X @ Sw, banded.  X^T rows [0,128) only reach out cols
            # [0, 128+bw) and rows [128,256) only cols [128-bw, 256), so per
            # image two K=128 matmuls of N=128+bw cover everything (the
            # straddle cols [lo,hi) accumulate from both); cols streamed drop
            # from 512 to 2*(128+bw) per (hb, bi).
            def xts(wb, bi):
                base = bi * 256 + hb * 128
                return xt[wb][:, base : base + 128]

            for bi in range(2):
                nc.tensor.matmul(
                    po[hb][:, bi * 256 : bi * 256 + hi],
                    lhsT=xts(0, bi),
                    rhs=mat[:, 384 : 384 + hi],
                    start=False,
                    stop=False,
                )
                nc.tensor.matmul(
                    po[hb][:, bi * 256 + lo : bi * 256 + 256],
                    lhsT=xts(1, bi),
                    rhs=mat[:, 384 + hi : 384 + 2 * hi],
                    start=False,
                    stop=(bi == 1),
                )

            # evacuate once the group is complete
            if hb == 0:
                nc.vector.tensor_copy(oh[0][:, p * 512 : p * 512 + 512], po[0][:])
            else:
                nc.scalar.copy(oh[1][:, p * 512 : p * 512 + 512], po[1][:])
            if hb == 1:
                del pairs[g], outs[g]
                if c == C_LOC - 1:
                    # last channel: store per pair so the tail is one chunk,
                    # not a whole channel; use sync (idle by now) + scalar so
                    # the slow gpsimd software-DGE drain is off the tail
                    nc.sync.dma_start(
                        o_d[c, 0, :, 2 * p : 2 * p + 2, :],
                        oh[0][:, p * 512 : p * 512 + 512],
                    )
                    nc.scalar.dma_start(
                        o_d[c, 1, :, 2 * p : 2 * p + 2, :],
                        oh[1][:, p * 512 : p * 512 + 512],
                    )
                elif p == 3:  # channel done: store (off the SP engine)
                    nc.gpsimd.dma_start(o_d[c, 0], oh[0][:])
                    nc.scalar.dma_start(o_d[c, 1], oh[1][:])

        # software pipeline: pair g's transposes are interleaved between pair
        # g-1's two matmul groups, so TensorE always has real matmuls in every
        # HAM window and the X^T evac latency is hidden one pair ahead.
        for g in range(n_pairs + 1):
            if g < n_pairs:
                c, p = divmod(g, 4)
                if p == 0:
                    start_channel(c)
                tr_half(g, 0)
            if g > 0:
                terms_group(g - 1, 0)
            if g < n_pairs:
                tr_half(g, 1)
            if g > 0:
                terms_group(g - 1, 1)


def _build_program(bw):
    global _PROGRAM
    if _PROGRAM is not None and _PROGRAM[0] == bw:
        return _PROGRAM[1]
    nc = bacc.Bacc("TRN2", target_bir_lowering=False, debug=False, num_devices=N_CORES)
    # DMA-native layouts (host pre-shuffles): x/out as [c, hb, h, b, w] so a
    # [128, 2048] tile load/store is contiguous 4KB per partition; mats as
    # [c, p, m, f] so a [128, 1024] tile load is contiguous 2KB per partition.
    x_d = nc.dram_tensor("x_sh", [C_LOC, 2, 128, B, W], BF16, kind="ExternalInput").ap()
    m_d = nc.dram_tensor(
        "mats", [C_LOC, 128, 640 + 2 * bw], BF16, kind="ExternalInput"
    ).ap()
    i_d = nc.dram_tensor("ident", [128, 128], BF16, kind="ExternalInput").ap()
    o_d = nc.dram_tensor("out_sh", [C_LOC, 2, 128, B, W], BF16, kind="ExternalOutput").ap()
    with tile.TileContext(nc) as tc:
        _emit(tc, x_d, m_d, i_d, o_d, bw)
    nc.compile()
    _PROGRAM = (bw, nc)
    return nc


def _eff_coeffs(taps, r):
    """taps: [k, C] per-tap depthwise weights -> dict integer_shift -> coeff[C].

    Mirrors the reference: pos = coord + off*r (f32), i0 = floor(pos),
    frac = pos - i0; both are constant per tap since coord is integral.
    """
    r_val = max(float(np.float32(r)), 1.0)
    k = taps.shape[0]
    pad = k // 2
    coeffs = {}
    for i, off in enumerate(range(-pad, pad + 1)):
        pos = np.float32(off * np.float32(r_val))
        s0 = int(np.floor(pos))
        f = float(np.float32(pos)) - s0
        for s, cmul in ((s0, 1.0 - f), (s0 + 1, f)):
            if cmul != 0.0:
                acc = coeffs.setdefault(s, np.zeros(taps.shape[1], np.float64))
                acc += cmul * taps[i].astype(np.float64)
    return coeffs


def _build_mats(weight_h, weight_w, r, bw):
    """Host-build the compressed per-channel banded-matrix strip.

    Returns [C, 128, 640+2*bw] f32, per channel:
      cols [0,128):        MhT diagonal block (Toeplitz: same for both
                           (kb,hb)=(0,0) and (1,1)), identity folded in
      cols [128,256):      MhT[128:256, 0:128]  (corner block for hb=0)
      cols [256,384):      MhT[0:128, 128:256]  (corner block for hb=1)
      cols [384,384+hi):   Sw[0:128, 0:hi)       (hi = 128+bw)
      cols [384+hi,...):   Sw[128:256, lo:256)   (lo = 128-bw), hi cols
    where MhT[h+s, h] = A[c, s] (+I) and Sw[w+t, w] = B[c, t].
    """
    lo, hi = 128 - bw, 128 + bw
    ch = _eff_coeffs(weight_h[:, 0, :, 0].T, r)
    cw = _eff_coeffs(weight_w[:, 0, 0, :].T, r)
    mh_t = np.zeros((C, H, H), np.float64)
    mh_t[:, np.arange(H), np.arange(H)] = 1.0
    for s, coef in ch.items():
        i = np.arange(max(0, s), H + min(0, s))
        mh_t[:, i, i - s] += coef[:, None]
    sw = np.zeros((C, W, W), np.float64)
    for t, coef in cw.items():
        i = np.arange(max(0, t), W + min(0, t))
        sw[:, i, i - t] += coef[:, None]
    mats = np.zeros((C, 128, 640 + 2 * bw), np.float32)
    mats[:, :, 0:128] = mh_t[:, 0:128, 0:128]
    mats[:, :, 128:256] = mh_t[:, 128:256, 0:128]
    mats[:, :, 256:384] = mh_t[:, 0:128, 128:256]
    mats[:, :, 384 : 384 + hi] = sw[:, 0:128, 0:hi]
    mats[:, :, 384 + hi : 384 + 2 * hi] = sw[:, 128:256, lo:256]
    return mats


def kernel(**inputs):
    global LAST_RESULTS
    x = np.ascontiguousarray(np.asarray(inputs["x"], dtype=np.float32))
    weight_h = np.asarray(inputs["weight_h"], dtype=np.float32)
    weight_w = np.asarray(inputs["weight_w"], dtype=np.float32)
    r = np.asarray(inputs["r"], dtype=np.float32)
    assert x.shape == (B, C, H, W), x.shape

    import ml_dtypes

    bf16 = ml_dtypes.bfloat16
    # band halfwidth: max integer shift is floor(3*r)+1 (bilinear upper tap)
    r_val = max(float(np.float32(r)), 1.0)
    bw = int(np.floor(3.0 * r_val)) + 1
    assert 1 <= bw <= 60, bw

    mats = _build_mats(weight_h, weight_w, r, bw)  # [C, 128, 640+2bw]
    mats = np.ascontiguousarray(mats.astype(bf16))
    ident = np.ascontiguousarray(np.eye(128, dtype=bf16))

    # [B, C, H, W] -> per-shard [C_LOC, 2(hb), 128(h), B, W] (DMA-native)
    xs = x.astype(bf16).transpose(1, 2, 0, 3).reshape(C, 2, 128, B, W)

    nc = _build_program(bw)
    in_maps = [
        {
            "x_sh": np.ascontiguousarray(xs[i * C_LOC : (i + 1) * C_LOC]),
            "mats": np.ascontiguousarray(mats[i * C_LOC : (i + 1) * C_LOC]),
            "ident": ident,
        }
        for i in range(N_CORES)
    ]
    res = run_bass_kernel_spmd(nc, in_maps, list(range(N_CORES)))
    LAST_RESULTS = res
    # [C_LOC, 2, 128, B, W] bf16 per core -> [B, C, H, W] f32
    o = np.concatenate(
        [np.asarray(res.results[i]["out_sh"]) for i in range(N_CORES)], axis=0
    )
    out = o.reshape(C, H, B, W).transpose(2, 0, 1, 3).astype(np.float32)
    return np.ascontiguousarray(out)

